# revision 1
# baseline (speedup 1.0000x reference)
"""CTRNN scan kernel for Trainium2 (8 NeuronCores, batch-sharded data parallel).

Problem: T=512, B=128, I=256, H=512, alpha=0.2.
    xin = einsum('tbi,hi->tbh', x, W_in)
    h_{t+1} = relu(h_t*(1-a) + (xin_t + h_t @ W_hh.T)*a)

Key algebraic fold: h*(1-a) + a*(h @ W_hh.T) = h @ M  with
    M = a*W_hh.T + (1-a)*I
so each scan step is ONE matmul plus relu(psum + a*xin_t).

Per-core layout (core c owns batch slice j in [16c, 16c+16)):
  - State kept transposed+packed: hT[p, 4j+m] = h[j, 128m+p]  (fp16, [128,64])
    so the matmul runs in "transposed orientation":
      hT_new[Hm block] = sum_k M[Hk, Hm].T @ hT[Hk block]
    with M tiles as stationary (lhsT) fp16 weights and hT slices as the
    16-column moving operand. The relu output feeds the next step with no
    transposes in the recurrence.
  - Input projection precomputed on-device in 16 chunks of 512 columns
    (N=512 matmuls, fp16 operands, f32 psum) into SBUF-resident f32 xinT.
  - Output drain: two consecutive steps pack one [128,128] fp16 tile which a
    DMA-transpose (16x128 xbar tiles) flips into natural [16,512] layout that
    is exactly contiguous in DRAM; a DVE copy upcasts fp16->f32.
"""

import numpy as np

T, B, I, H = 512, 128, 256, 512
NCORES = 8
BS = B // NCORES  # 16
ALPHA = np.float32(20.0 / 100.0)
ONE_MINUS_ALPHA = np.float32(1.0 - 20.0 / 100.0)

NCHUNK = 16
CH = T * BS // NCHUNK  # 512 t*b columns per precompute chunk = 32 steps

_CACHE = {}


def _build_program():
    import concourse.bacc as bacc
    import concourse.mybir as mybir
    from concourse.tile import TileContext

    f16 = mybir.dt.float16
    f32 = mybir.dt.float32

    nc = bacc.Bacc("TRN2", target_bir_lowering=False, debug=False,
                   num_devices=NCORES)

    xT_d = nc.dram_tensor("xT", [I, T * BS], f16, kind="ExternalInput")
    mhh_d = nc.dram_tensor("mhh", [H, H], f16, kind="ExternalInput")
    winp_d = nc.dram_tensor("winp", [I, H], f16, kind="ExternalInput")
    hT0_d = nc.dram_tensor("hT0", [128, 4 * BS], f16, kind="ExternalInput")
    out_d = nc.dram_tensor("out", [T, BS, H], f32, kind="ExternalOutput")

    add = mybir.AluOpType.add

    with TileContext(nc) as tc:
        with tc.tile_pool(name="wpool", bufs=1) as wpool:
            # --- persistent weights ---
            mhh_sb = []
            for k in range(4):
                mk = wpool.tile([128, H], f16, name=f"mhh{k}", tag=f"mhh{k}")
                nc.sync.dma_start(mk[:], mhh_d[128 * k:128 * (k + 1), :])
                mhh_sb.append(mk)
            winp_sb = []
            for k in range(2):
                wk = wpool.tile([128, H], f16, name=f"winp{k}", tag=f"winp{k}")
                nc.sync.dma_start(wk[:], winp_d[128 * k:128 * (k + 1), :])
                winp_sb.append(wk)
            hT_init = wpool.tile([128, 4 * BS], f16, name="hT_init",
                                 tag="hT_init")
            nc.sync.dma_start(hT_init[:], hT0_d[:])

            # --- phase 1: xinT = (a*W_in) @ x^T, chunked, SBUF-resident f32 ---
            xin_tiles = []
            xT_v = xT_d.rearrange("(two p) n -> p two n", p=128)
            with tc.tile_pool(name="pre_psum", bufs=8, space="PSUM") as pre_psum, \
                 tc.tile_pool(name="xt_pool", bufs=3) as xt_pool:
                for c in range(NCHUNK):
                    xt = xt_pool.tile([128, 2, CH], f16, name="xt", tag="xt")
                    nc.sync.dma_start(xt[:], xT_v[:, :, c * CH:(c + 1) * CH])
                    xin_c = wpool.tile([128, 4, CH], f32, name=f"xin{c}",
                                       tag=f"xin{c}")
                    for m in range(4):
                        ps = pre_psum.tile([128, CH], f32, name="ps", tag="ps")
                        for k in range(2):
                            nc.tensor.matmul(
                                ps[:],
                                winp_sb[k][:, 128 * m:128 * (m + 1)],
                                xt[:, k, :],
                                start=(k == 0), stop=(k == 1),
                            )
                        nc.any.tensor_copy(xin_c[:, m, :], ps[:])
                    xin_tiles.append(xin_c)

            # --- phase 2: the scan ---
            out_flat = out_d.rearrange("t b h -> (t b h)")
            pair_tiles = {}
            natT = None
            with tc.tile_pool(name="scan_psum", bufs=4, space="PSUM") as scan_psum, \
                 tc.tile_pool(name="pair_pool", bufs=6) as pair_pool, \
                 tc.tile_pool(name="nat_pool", bufs=2) as nat_pool, \
                 tc.tile_pool(name="natf_pool", bufs=2) as natf_pool:
                for t in range(T):
                    # moving operand view of hT(t): [128, k(4), j(16)]
                    if t == 0:
                        rhs_v = hT_init.rearrange("p (j four) -> p four j",
                                                  four=4)
                    else:
                        u, hh = divmod(t - 1, 2)
                        rhs_v = pair_tiles[u].rearrange(
                            "p (half j four) -> p half four j",
                            half=2, four=4)[:, hh]

                    ps = scan_psum.tile([128, 4 * BS], f32, name="sps",
                                        tag="sps")
                    for m in range(4):
                        for k in range(4):
                            nc.tensor.matmul(
                                ps[:, BS * m:BS * (m + 1)],
                                mhh_sb[k][:, 128 * m:128 * (m + 1)],
                                rhs_v[:, k, :],
                                start=(k == 0), stop=(k == 3),
                            )

                    # hT(t+1) = relu(psum + xin_t), fp16, packed into pair
                    u2, h2 = divmod(t, 2)
                    if h2 == 0:
                        pair_tiles[u2] = pair_pool.tile([128, 128], f16,
                                                        name="pair", tag="pair")
                    pair = pair_tiles[u2]
                    pair_v = pair.rearrange("p (half j four) -> p half four j",
                                            half=2, four=4)
                    c, tt = divmod(t, 32)
                    for m in range(4):
                        dst = pair_v[:, h2, m, :]
                        nc.vector.tensor_tensor(
                            dst, ps[:, BS * m:BS * (m + 1)],
                            xin_tiles[c][:, m, BS * tt:BS * (tt + 1)], add)
                        nc.vector.tensor_scalar_max(dst, dst, 0.0)

                    # drain: pair complete after odd step
                    if h2 == 1:
                        natidx = u2 % 4
                        if natidx == 0:
                            natT = nat_pool.tile([128, 4, 128], f16,
                                                 name="natT", tag="natT")
                        nc.sync.dma_start(natT[:, natidx, :], pair[:],
                                          transpose=True)
                        if natidx == 3:
                            v = u2 // 4
                            natF = natf_pool.tile([128, 4, 128], f32,
                                                  name="natF", tag="natF")
                            nc.vector.tensor_copy(natF[:], natT[:])
                            dram_ap = out_flat[
                                v * 8 * BS * H:(v + 1) * 8 * BS * H
                            ].rearrange("(c r p) -> r c p", c=4, r=128)
                            nc.sync.dma_start(dram_ap, natF[:])

    nc.finalize()
    return nc


def _prep_inputs(x, hidden, W_in, W_hh):
    """Host-side weight prep + per-core sharding/layout (no heavy FLOPs)."""
    Mhh = (ALPHA * W_hh.T
           + ONE_MINUS_ALPHA * np.eye(H, dtype=np.float32)).astype(np.float16)
    WinP = (ALPHA * W_in.T).astype(np.float16)  # [I, H]
    in_maps = []
    for cidx in range(NCORES):
        sl = slice(cidx * BS, (cidx + 1) * BS)
        # xT[i, t*BS + j] = x[t, sl.start + j, i]
        xT = np.ascontiguousarray(
            x[:, sl, :].transpose(2, 0, 1).reshape(I, T * BS)
        ).astype(np.float16)
        # hT0[p, 4j+m] = hidden[sl.start + j, 128m + p]
        hT0 = np.ascontiguousarray(
            hidden[sl].reshape(BS, 4, 128).transpose(2, 0, 1).reshape(128, 4 * BS)
        ).astype(np.float16)
        in_maps.append({"xT": xT, "mhh": Mhh, "winp": WinP, "hT0": hT0})
    return in_maps


def kernel(x, hidden, W_in, W_hh):
    from concourse.bass_utils import run_bass_kernel_spmd

    if "nc" not in _CACHE:
        _CACHE["nc"] = _build_program()
    nc = _CACHE["nc"]

    in_maps = _prep_inputs(np.asarray(x), np.asarray(hidden),
                           np.asarray(W_in), np.asarray(W_hh))
    res = run_bass_kernel_spmd(nc, in_maps, list(range(NCORES)))
    out = np.concatenate([res.results[c]["out"] for c in range(NCORES)],
                         axis=1)  # [T, B, H]
    return out, out[-1].copy()


# revision 2
# speedup vs baseline: 5581.7980x; 5581.7980x over previous
"""CTRNN scan kernel for Trainium2 (8 NeuronCores, batch-sharded data parallel).

Problem: T=512, B=128, I=256, H=512, alpha=0.2.
    xin = einsum('tbi,hi->tbh', x, W_in)
    h_{t+1} = relu(h_t*(1-a) + (xin_t + h_t @ W_hh.T)*a)

Key algebraic fold: h*(1-a) + a*(h @ W_hh.T) = h @ M  with
    M = a*W_hh.T + (1-a)*I
so each scan step is ONE matmul plus relu(psum + a*xin_t).

Per-core layout (core c owns batch slice j in [16c, 16c+16)):
  - State kept transposed+packed: hT[p, 4j+m] = h[j, 128m+p]  (fp16, [128,64])
    so the matmul runs in "transposed orientation":
      hT_new[Hm block] = sum_k M[Hk, Hm].T @ hT[Hk block]
    with M tiles as stationary (lhsT) fp16 weights and hT slices as the
    16-column moving operand. The relu output feeds the next step with no
    transposes in the recurrence.
  - a*xin_t is DVE-copied into the PSUM bank before the step's matmuls, which
    then run in accumulate mode (start=False, has_written bits pre-set by a
    one-time priming matmul per bank), so the serial chain per step is just
    matmuls -> one fused relu DVE op.
  - Input projection precomputed on-device in 16 chunks of 512 columns
    (N=512 matmuls, fp16 operands, f32 psum) into SBUF-resident f32 xinT.
  - Output drain: two consecutive steps pack one [128,128] fp16 tile which a
    DMA-transpose (16x128 xbar tiles) flips into natural [16,512] layout that
    is exactly contiguous in DRAM; a DVE copy upcasts fp16->f32.
"""

import numpy as np

T, B, I, H = 512, 128, 256, 512
NCORES = 8
BS = B // NCORES  # 16
ALPHA = np.float32(20.0 / 100.0)
ONE_MINUS_ALPHA = np.float32(1.0 - 20.0 / 100.0)

NCHUNK = 16
CH = T * BS // NCHUNK  # 512 t*b columns per precompute chunk = 32 steps

_CACHE = {}


def _build_program(timing_reps=None):
    """Build the Bass program. timing_reps=K wraps the whole computation in a
    device-side For_i loop with outputs redirected to internal DRAM (tiny
    token output) so repeated-execution wall-clock deltas isolate exec time."""
    import concourse.bacc as bacc
    import concourse.mybir as mybir
    from concourse.tile import TileContext

    f16 = mybir.dt.float16
    f32 = mybir.dt.float32

    nc = bacc.Bacc("TRN2", target_bir_lowering=False, debug=False,
                   num_devices=NCORES)

    xT_d = nc.dram_tensor("xT", [I, T * BS], f16, kind="ExternalInput")
    mhh_d = nc.dram_tensor("mhh", [H, H], f16, kind="ExternalInput")
    winp_d = nc.dram_tensor("winp", [I, H], f16, kind="ExternalInput")
    hT0_d = nc.dram_tensor("hT0", [128, 4 * BS], f16, kind="ExternalInput")
    if timing_reps is None:
        out_d = nc.dram_tensor("out", [T, BS, H], f32, kind="ExternalOutput")
    else:
        out_d = nc.dram_tensor("out_scratch", [T, BS, H], f32)  # internal
        tok_d = nc.dram_tensor("tok", [128, 1], f32, kind="ExternalOutput")

    add = mybir.AluOpType.add

    with TileContext(nc) as tc:
        with tc.tile_pool(name="wpool", bufs=1) as wpool:
            # --- persistent weights ---
            mhh_sb = []
            for k in range(4):
                mk = wpool.tile([128, H], f16, name=f"mhh{k}", tag=f"mhh{k}")
                nc.sync.dma_start(mk[:], mhh_d[128 * k:128 * (k + 1), :])
                mhh_sb.append(mk)
            winp_sb = []
            for k in range(2):
                wk = wpool.tile([128, H], f16, name=f"winp{k}", tag=f"winp{k}")
                nc.sync.dma_start(wk[:], winp_d[128 * k:128 * (k + 1), :])
                winp_sb.append(wk)
            hT_init = wpool.tile([128, 4 * BS], f16, name="hT_init",
                                 tag="hT_init")
            nc.sync.dma_start(hT_init[:], hT0_d[:])

            import contextlib

            @contextlib.contextmanager
            def _maybe_loop():
                if timing_reps is None:
                    yield
                else:
                    with tc.For_i(0, timing_reps, 1):
                        yield

            with _maybe_loop():
                # --- phase 1: xinT = (a*W_in) @ x^T, SBUF-resident f32 ---
                xin_tiles = []
                xT_v = xT_d.rearrange("(two p) n -> p two n", p=128)
                with tc.tile_pool(name="pre_psum", bufs=8, space="PSUM") as pre_psum, \
                     tc.tile_pool(name="xt_pool", bufs=3) as xt_pool:
                    for c in range(NCHUNK):
                        xt = xt_pool.tile([128, 2, CH], f16, name="xt",
                                          tag="xt")
                        nc.sync.dma_start(xt[:],
                                          xT_v[:, :, c * CH:(c + 1) * CH])
                        xin_c = wpool.tile([128, 4, CH], f32, name=f"xin{c}",
                                           tag=f"xin{c}")
                        for m in range(4):
                            ps = pre_psum.tile([128, CH], f32, name="ps",
                                               tag="ps")
                            for k in range(2):
                                nc.tensor.matmul(
                                    ps[:],
                                    winp_sb[k][:, 128 * m:128 * (m + 1)],
                                    xt[:, k, :],
                                    start=(k == 0), stop=(k == 1),
                                )
                            nc.any.tensor_copy(xin_c[:, m, :], ps[:])
                        xin_tiles.append(xin_c)

                # --- phase 2: the scan ---
                out_flat = out_d.rearrange("t b h -> (t b h)")
                pair_tiles = {}
                natT = None
                with tc.tile_pool(name="scan_psum", bufs=4, space="PSUM") as scan_psum, \
                     tc.tile_pool(name="pair_pool", bufs=6) as pair_pool, \
                     tc.tile_pool(name="nat_pool", bufs=2) as nat_pool, \
                     tc.tile_pool(name="natf_pool", bufs=2) as natf_pool:
                    # Prime the 4 psum banks: one throwaway matmul each sets
                    # every element's has_written bit so the per-step matmuls
                    # can run in accumulate mode on DVE-preloaded xin.
                    prime = []
                    for _ in range(4):
                        pp = scan_psum.tile([128, 4 * BS], f32, name="sps",
                                            tag="sps")
                        nc.tensor.matmul(pp[:], mhh_sb[0][:, 0:128],
                                         hT_init[:], start=True, stop=True)
                        prime.append(pp)

                    for t in range(T):
                        # moving operand view of hT(t): [128, k(4), j(16)]
                        if t == 0:
                            rhs_v = hT_init.rearrange("p (j four) -> p four j",
                                                      four=4)
                        else:
                            u, hh = divmod(t - 1, 2)
                            rhs_v = pair_tiles[u].rearrange(
                                "p (half j four) -> p half four j",
                                half=2, four=4)[:, hh]

                        if t < 4:
                            ps = prime[t]
                        else:
                            ps = scan_psum.tile([128, 4 * BS], f32,
                                                name="sps", tag="sps")
                        c, tt = divmod(t, 32)
                        # preload a*xin_t into psum (not on the serial chain)
                        nc.vector.tensor_copy(
                            ps.rearrange("p (m j) -> p m j", m=4),
                            xin_tiles[c][:, :, BS * tt:BS * (tt + 1)])
                        for m in range(4):
                            for k in range(4):
                                nc.tensor.matmul(
                                    ps[:, BS * m:BS * (m + 1)],
                                    mhh_sb[k][:, 128 * m:128 * (m + 1)],
                                    rhs_v[:, k, :],
                                    start=False, stop=(k == 3),
                                    skip_group_check=True,
                                )

                        # hT(t+1) = relu(psum), fp16, packed into pair tile
                        u2, h2 = divmod(t, 2)
                        if h2 == 0:
                            pair_tiles[u2] = pair_pool.tile(
                                [128, 128], f16, name="pair", tag="pair")
                        pair = pair_tiles[u2]
                        pair_v = pair.rearrange(
                            "p (half j four) -> p half four j",
                            half=2, four=4)
                        nc.vector.tensor_scalar_max(
                            pair_v[:, h2],
                            ps.rearrange("p (m j) -> p m j", m=4), 0.0)

                        # drain: pair complete after odd step
                        if h2 == 1:
                            natidx = u2 % 4
                            if natidx == 0:
                                natT = nat_pool.tile([128, 4, 128], f16,
                                                     name="natT", tag="natT")
                            nc.sync.dma_start(natT[:, natidx, :], pair[:],
                                              transpose=True)
                            if natidx == 3:
                                v = u2 // 4
                                natF = natf_pool.tile([128, 4, 128], f32,
                                                      name="natF", tag="natF")
                                nc.vector.tensor_copy(natF[:], natT[:])
                                dram_ap = out_flat[
                                    v * 8 * BS * H:(v + 1) * 8 * BS * H
                                ].rearrange("(c r p) -> r c p", c=4, r=128)
                                nc.sync.dma_start(dram_ap, natF[:])

            if timing_reps is not None:
                with tc.tile_pool(name="tokp", bufs=1) as tokp:
                    tok = tokp.tile([128, 1], f32, name="tok", tag="tok")
                    nc.vector.memset(tok[:], 1.0)
                    nc.sync.dma_start(tok_d[:], tok[:])

    nc.finalize()
    return nc


def _prep_inputs(x, hidden, W_in, W_hh):
    """Host-side weight prep + per-core sharding/layout (no heavy FLOPs)."""
    Mhh = (ALPHA * W_hh.T
           + ONE_MINUS_ALPHA * np.eye(H, dtype=np.float32)).astype(np.float16)
    WinP = (ALPHA * W_in.T).astype(np.float16)  # [I, H]
    in_maps = []
    for cidx in range(NCORES):
        sl = slice(cidx * BS, (cidx + 1) * BS)
        # xT[i, t*BS + j] = x[t, sl.start + j, i]
        xT = np.ascontiguousarray(
            x[:, sl, :].transpose(2, 0, 1).reshape(I, T * BS)
        ).astype(np.float16)
        # hT0[p, 4j+m] = hidden[sl.start + j, 128m + p]
        hT0 = np.ascontiguousarray(
            hidden[sl].reshape(BS, 4, 128).transpose(2, 0, 1).reshape(128, 4 * BS)
        ).astype(np.float16)
        in_maps.append({"xT": xT, "mhh": Mhh, "winp": WinP, "hT0": hT0})
    return in_maps


def kernel(x, hidden, W_in, W_hh):
    from concourse.bass_utils import run_bass_kernel_spmd

    if "nc" not in _CACHE:
        _CACHE["nc"] = _build_program()
    nc = _CACHE["nc"]

    in_maps = _prep_inputs(np.asarray(x), np.asarray(hidden),
                           np.asarray(W_in), np.asarray(W_hh))
    res = run_bass_kernel_spmd(nc, in_maps, list(range(NCORES)))
    out = np.concatenate([res.results[c]["out"] for c in range(NCORES)],
                         axis=1)  # [T, B, H]
    return out, out[-1].copy()


# revision 17
# speedup vs baseline: 10704.6343x; 1.9178x over previous
"""CTRNN scan kernel for Trainium2 (8 NeuronCores, batch-sharded data parallel).

Problem: T=512, B=128, I=256, H=512, alpha=0.2.
    xin = einsum('tbi,hi->tbh', x, W_in)
    h_{t+1} = relu(h_t*(1-a) + (xin_t + h_t @ W_hh.T)*a)

Key algebraic fold: h*(1-a) + a*(h @ W_hh.T) = h @ M  with
    M = a*W_hh.T + (1-a)*I
so each scan step is ONE matmul plus relu(psum + a*xin_t).

Per-core layout (core c owns batch slice j in [16c, 16c+16)):
  - State kept transposed+packed: hT[p, 4j+m] = h[j, 128m+p]  (fp16, [128,64])
    so the matmul runs in "transposed orientation":
      hT_new[Hm block] = sum_k M[Hk, Hm].T @ hT[Hk block]
    with M tiles as stationary (lhsT) fp16 weights and hT slices as the
    16-column moving operand. The relu output feeds the next step with no
    transposes in the recurrence.
  - a*xin_t is DVE-copied into the PSUM bank before the step's matmuls, which
    then run in accumulate mode (start=False, has_written bits pre-set by a
    one-time priming matmul per bank), so the serial chain per step is just
    matmuls -> one fused relu DVE op.
  - Input projection precomputed on-device in 16 chunks of 512 columns
    (N=512 matmuls, fp16 operands, f32 psum) into SBUF-resident f32 xinT.
  - Output drain: two consecutive steps pack one [128,128] fp16 tile which a
    DMA-transpose (16x128 xbar tiles) flips into natural [16,512] layout that
    is exactly contiguous in DRAM; a DVE copy upcasts fp16->f32.
"""

import numpy as np

T, B, I, H = 512, 128, 256, 512
NCORES = 8
BS = B // NCORES  # 16
ALPHA = np.float32(20.0 / 100.0)
ONE_MINUS_ALPHA = np.float32(1.0 - 20.0 / 100.0)

NCHUNK = 16
CH = T * BS // NCHUNK  # 512 t*b columns per precompute chunk = 32 steps

_CACHE = {}


def _build_program(timing_reps=None, abl_mms=True, abl_chain=True,
                   abl_drain=True, abl_preload=True, abl_precompute=True):
    """Build the Bass program. timing_reps=K wraps the whole computation in a
    device-side For_i loop with outputs redirected to internal DRAM (tiny
    token output) so repeated-execution wall-clock deltas isolate exec time.
    The abl_* flags carve out pieces for timing ablations (wrong results)."""
    import concourse.bacc as bacc
    import concourse.mybir as mybir
    from concourse.tile import TileContext

    f16 = mybir.dt.float16
    f32 = mybir.dt.float32

    nc = bacc.Bacc("TRN2", target_bir_lowering=False, debug=False,
                   num_devices=NCORES)

    xT_d = nc.dram_tensor("xT", [I, T * BS], f16, kind="ExternalInput")
    mhh_d = nc.dram_tensor("mhh", [H, H], f16, kind="ExternalInput")
    winp_d = nc.dram_tensor("winp", [I, H], f16, kind="ExternalInput")
    hT0_d = nc.dram_tensor("hT0", [128, 4 * BS], f16, kind="ExternalInput")
    eye_d = nc.dram_tensor("eye", [128, 128], f16, kind="ExternalInput")
    if timing_reps is None:
        out_d = nc.dram_tensor("out", [T, BS, H], f32, kind="ExternalOutput")
    else:
        out_d = nc.dram_tensor("out_scratch", [T, BS, H], f32)  # internal
        tok_d = nc.dram_tensor("tok", [128, 1], f32, kind="ExternalOutput")

    add = mybir.AluOpType.add

    with TileContext(nc) as tc:
        with tc.tile_pool(name="wpool", bufs=1) as wpool:
            # --- persistent weights ---
            mhh_sb = []
            for k in range(4):
                mk = wpool.tile([128, H], f16, name=f"mhh{k}", tag=f"mhh{k}")
                nc.sync.dma_start(mk[:], mhh_d[128 * k:128 * (k + 1), :])
                mhh_sb.append(mk)
            winp_sb = []
            for k in range(2):
                wk = wpool.tile([128, H], f16, name=f"winp{k}", tag=f"winp{k}")
                nc.sync.dma_start(wk[:], winp_d[128 * k:128 * (k + 1), :])
                winp_sb.append(wk)
            hT_init = wpool.tile([128, 4 * BS], f16, name="hT_init",
                                 tag="hT_init")
            nc.sync.dma_start(hT_init[:], hT0_d[:])
            eye_sb = wpool.tile([128, 128], f16, name="eye_sb", tag="eye_sb")
            nc.sync.dma_start(eye_sb[:], eye_d[:])

            import contextlib

            @contextlib.contextmanager
            def _maybe_loop():
                if timing_reps is None:
                    yield
                else:
                    with tc.For_i(0, timing_reps, 1):
                        yield

            STEPC = CH // BS  # scan steps covered per precompute chunk (32)
            pre_tiles = []
            if not abl_precompute:
                for c in range(NCHUNK):
                    xin_c = wpool.tile([128, STEPC, 4 * BS], f32,
                                       name=f"xin{c}", tag=f"xin{c}")
                    nc.vector.memset(xin_c[:], 0.0)
                    pre_tiles.append(xin_c)

            with _maybe_loop():
                # --- phase 1: xinT = (a*W_in) @ x^T, SBUF-resident f32 ---
                # Chunk layout [128, t, 64] with cols interleaved as 4j+m so
                # the per-step psum preload is a contiguous [128,64] copy.
                xin_tiles = list(pre_tiles)
                xT_v = xT_d.rearrange("(two p) n -> p two n", p=128)
                with tc.tile_pool(name="pre_psum", bufs=8, space="PSUM") as pre_psum, \
                     tc.tile_pool(name="xt_pool", bufs=3) as xt_pool:
                    for c in range(NCHUNK if abl_precompute else 0):
                        xin_c = wpool.tile([128, STEPC, 4 * BS], f32,
                                           name=f"xin{c}", tag=f"xin{c}")
                        xt = xt_pool.tile([128, 2, CH], f16, name="xt",
                                          tag="xt")
                        nc.sync.dma_start(xt[:],
                                          xT_v[:, :, c * CH:(c + 1) * CH])
                        for m in range(4):
                            ps = pre_psum.tile([128, CH], f32, name="ps",
                                               tag="ps")
                            for k in range(2):
                                nc.tensor.matmul(
                                    ps[:],
                                    winp_sb[k][:, 128 * m:128 * (m + 1)],
                                    xt[:, k, :],
                                    start=(k == 0), stop=(k == 1),
                                )
                            nc.any.tensor_copy(
                                xin_c[:, :, BS * m:BS * (m + 1)],
                                ps.rearrange("p (t j) -> p t j", j=BS))
                        xin_tiles.append(xin_c)

                # --- phase 2: the scan ---
                out_flat = out_d.rearrange("t b h -> (t b h)")
                pair_tiles = {}
                natF = None
                with tc.tile_pool(name="scan_psum", bufs=4, space="PSUM") as scan_psum, \
                     tc.tile_pool(name="tp_psum", bufs=2, space="PSUM") as tp_psum, \
                     tc.tile_pool(name="pair_pool", bufs=6) as pair_pool, \
                     tc.tile_pool(name="natf_pool", bufs=2) as natf_pool:
                    # Prime the 4 psum banks: one throwaway matmul each sets
                    # every element's has_written bit so the per-step matmuls
                    # can run in accumulate mode on DVE-preloaded xin.
                    prime = []
                    for _ in range(4):
                        pp = scan_psum.tile([128, 4 * BS], f32, name="sps",
                                            tag="sps")
                        nc.tensor.matmul(pp[:], mhh_sb[0][:, 0:128],
                                         hT_init[:], start=True, stop=True)
                        prime.append(pp)

                    for t in range(T):
                        # moving operand view of hT(t): [128, k(4), j(16)]
                        if t == 0 or not abl_chain:
                            rhs_v = hT_init.rearrange("p (j four) -> p four j",
                                                      four=4)
                        else:
                            u, hh = divmod(t - 1, 2)
                            rhs_v = pair_tiles[u].rearrange(
                                "p (half j four) -> p half four j",
                                half=2, four=4)[:, hh]

                        if t < 4:
                            ps = prime[t]
                        else:
                            ps = scan_psum.tile([128, 4 * BS], f32,
                                                name="sps", tag="sps")
                        c, tt = divmod(t, STEPC)
                        # preload a*xin_t into psum (not on the serial chain)
                        if abl_preload:
                            nc.vector.tensor_copy(ps[:],
                                                  xin_tiles[c][:, tt, :])
                        if abl_mms:
                            for m in range(4):
                                for k in range(4):
                                    nc.tensor.matmul(
                                        ps[:, BS * m:BS * (m + 1)],
                                        mhh_sb[k][:, 128 * m:128 * (m + 1)],
                                        rhs_v[:, k, :],
                                        start=(not abl_preload and k == 0),
                                        stop=(k == 3),
                                        skip_group_check=True,
                                    )

                        # hT(t+1) = relu(psum), fp16, packed into pair tile
                        u2, h2 = divmod(t, 2)
                        if h2 == 0:
                            pair_tiles[u2] = pair_pool.tile(
                                [128, 128], f16, name="pair", tag="pair")
                        pair = pair_tiles[u2]
                        pair_v = pair.rearrange(
                            "p (half j four) -> p half four j",
                            half=2, four=4)
                        nc.scalar.activation(
                            pair_v[:, h2],
                            ps.rearrange("p (m j) -> p m j", m=4),
                            mybir.ActivationFunctionType.Relu)

                        # drain: pair complete after odd step. PE-transpose
                        # the [128,128] pair (rows become 128*r + p -> the
                        # natural DRAM order), copy psum->sbuf f32, and DMA
                        # out every 4 pairs (8 steps).
                        if h2 == 1 and abl_drain:
                            natidx = u2 % 4
                            if natidx == 0:
                                natF = natf_pool.tile([128, 4, 128], f32,
                                                      name="natF", tag="natF")
                            tp = tp_psum.tile([128, 128], f16, name="tp",
                                              tag="tp")
                            nc.tensor.transpose(tp[:], pair[:], eye_sb[:])
                            nc.vector.tensor_copy(natF[:, natidx, :], tp[:])
                            if natidx == 3:
                                v = u2 // 4
                                dram_ap = out_flat[
                                    v * 8 * BS * H:(v + 1) * 8 * BS * H
                                ].rearrange("(c r p) -> r c p", c=4, r=128)
                                nc.sync.dma_start(dram_ap, natF[:])

            if timing_reps is not None:
                with tc.tile_pool(name="tokp", bufs=1) as tokp:
                    tok = tokp.tile([128, 1], f32, name="tok", tag="tok")
                    nc.vector.memset(tok[:], 1.0)
                    nc.sync.dma_start(tok_d[:], tok[:])

    nc.finalize()
    return nc


def _prep_inputs(x, hidden, W_in, W_hh):
    """Host-side weight prep + per-core sharding/layout (no heavy FLOPs)."""
    Mhh = (ALPHA * W_hh.T
           + ONE_MINUS_ALPHA * np.eye(H, dtype=np.float32)).astype(np.float16)
    WinP = (ALPHA * W_in.T).astype(np.float16)  # [I, H]
    EYE = np.eye(128, dtype=np.float16)
    in_maps = []
    for cidx in range(NCORES):
        sl = slice(cidx * BS, (cidx + 1) * BS)
        # xT[i, t*BS + j] = x[t, sl.start + j, i]
        xT = np.ascontiguousarray(
            x[:, sl, :].transpose(2, 0, 1).reshape(I, T * BS)
        ).astype(np.float16)
        # hT0[p, 4j+m] = hidden[sl.start + j, 128m + p]
        hT0 = np.ascontiguousarray(
            hidden[sl].reshape(BS, 4, 128).transpose(2, 0, 1).reshape(128, 4 * BS)
        ).astype(np.float16)
        in_maps.append({"xT": xT, "mhh": Mhh, "winp": WinP, "hT0": hT0,
                        "eye": EYE})
    return in_maps


def kernel(x, hidden, W_in, W_hh):
    from concourse.bass_utils import run_bass_kernel_spmd

    if "nc" not in _CACHE:
        _CACHE["nc"] = _build_program()
    nc = _CACHE["nc"]

    in_maps = _prep_inputs(np.asarray(x), np.asarray(hidden),
                           np.asarray(W_in), np.asarray(W_hh))
    res = run_bass_kernel_spmd(nc, in_maps, list(range(NCORES)))
    out = np.concatenate([res.results[c]["out"] for c in range(NCORES)],
                         axis=1)  # [T, B, H]
    return out, out[-1].copy()
